# revision 10
# baseline (speedup 1.0000x reference)
"""AdaptiveFeatureFusion Trainium2 kernel (8 NeuronCores, data-parallel).

Math rewrite: softmax over 2 logits -> sigmoid of the logit difference.
  delta[b] = sum_ij v[b,i] * (W0 - W1)[i,j] * s[b,j] + (b0 - b1)
           = rowsum((v @ W0 - v @ W1) * s) + (b0 - b1)
  a[b]     = sigmoid(delta[b])
  out[b,:] = a[b] * v[b,:] + (1 - a[b]) * s[b,:] = s + a*(v - s)

Sharding: batch dim (512) split across 8 cores (64 rows each); the
(2, 768*768) fc weight is replicated and streamed through each core.
Per-core compute: vT via PE transposes, U_k = v_rows @ W_k on TensorE
(float32r, full rate at N>=384), the dot/sigmoid/fusion epilogue on
Vector/Scalar engines.
"""

import os
import sys

for _p in ("/opt/trn_rl_repo", "/opt/pypackages"):
    if os.path.isdir(_p) and _p not in sys.path:
        sys.path.append(_p)

import numpy as np

B = 512
D = 768
NCORES = 8
BPC = B // NCORES  # 64 rows per core
NT = D // 128  # 6 i-tiles
NH = 2  # N halves of 384
WCHUNKS = 3  # DMA chunks per W row (2 i-tiles each)

_CACHE = {}


def _build(mm_mode="f32"):
    from concourse import bacc, mybir
    from concourse import tile

    f32 = mybir.dt.float32
    f32r = mybir.dt.float32r
    bf16 = mybir.dt.bfloat16
    AluOp = mybir.AluOpType
    Act = mybir.ActivationFunctionType

    nc = bacc.Bacc(None, target_bir_lowering=False)

    v_ext = nc.declare_dram_parameter("v_x", [BPC, D], f32, isOutput=False)
    s_ext = nc.declare_dram_parameter("s_x", [BPC, D], f32, isOutput=False)
    w_ext = nc.declare_dram_parameter("fc_w", [2, D * D], f32, isOutput=False)
    b_ext = nc.declare_dram_parameter("fc_b", [2], f32, isOutput=False)
    id_ext = nc.declare_dram_parameter("ident", [BPC, BPC], f32, isOutput=False)
    out_ext = nc.declare_dram_parameter("out", [BPC, D], f32, isOutput=True)

    with tile.TileContext(nc) as tc:
        with (
            tc.tile_pool(name="sb", bufs=1) as sb,
            tc.tile_pool(name="ps", bufs=1, space="PSUM") as ps,
            tc.tile_pool(name="tps", bufs=2, space="PSUM") as tps,
        ):
            # --- input DMAs -------------------------------------------------
            v_sb = sb.tile([BPC, D], f32, tag="v")
            nc.sync.dma_start(out=v_sb[:, :], in_=v_ext[:, :])
            s_sb = sb.tile([BPC, D], f32, tag="s")
            nc.sync.dma_start(out=s_sb[:, :], in_=s_ext[:, :])
            id_sb = sb.tile([BPC, BPC], f32, tag="id")
            nc.sync.dma_start(out=id_sb[:, :], in_=id_ext[:, :])
            fcb_sb = sb.tile([1, 2], f32, tag="fcb")
            nc.sync.dma_start(out=fcb_sb[:, :], in_=b_ext.ap().unsqueeze(0))

            # W: per k one SBUF tile (128, NT*768), free dim is (i_tile, j)
            # i_tile-major; DMA'd in WCHUNKS chunks of 2 i-tiles (786 KB).
            w_sb = []
            for k in range(2):
                w_sb.append(sb.tile([128, NT * D], f32, tag=f"w{k}", name=f"w{k}"))
            rows_per_chunk = 256 * D  # 2 i-tiles x 128 partitions x 768
            for k in range(2):
                for c in range(WCHUNKS):
                    src = w_ext[k, c * rows_per_chunk : (c + 1) * rows_per_chunk]
                    src = src.rearrange("(t p j) -> p t j", t=2, p=128, j=D)
                    dst = w_sb[k][:, c * 2 * D : (c + 1) * 2 * D].rearrange(
                        "p (t j) -> p t j", t=2, j=D
                    )
                    nc.sync.dma_start(out=dst, in_=src)

            # --- vT via PE transposes (plus negated copy for the k=1
            # matmuls, so U0 - U1 accumulates directly in PSUM) -------------
            vt_sb = sb.tile([128, NT * BPC], f32, tag="vt")
            vtn_sb = sb.tile([128, NT * BPC], f32, tag="vtn")
            for t in range(NT):
                tp = tps.tile([128, BPC], f32, tag="tp")
                nc.tensor.transpose(
                    tp[:, :], v_sb[:, t * 128 : (t + 1) * 128], id_sb[:, :]
                )
                nc.vector.tensor_copy(vt_sb[:, t * BPC : (t + 1) * BPC], tp[:, :])
                nc.vector.tensor_scalar_mul(
                    vtn_sb[:, t * BPC : (t + 1) * BPC], tp[:, :], -1.0
                )

            # --- bias difference broadcast to all partitions ---------------
            ones_sb = sb.tile([1, BPC], f32, tag="ones")
            nc.gpsimd.memset(ones_sb[:, :], 1.0)
            bd_sb = sb.tile([1, 1], f32, tag="bd")
            nc.vector.tensor_sub(bd_sb[:, :], fcb_sb[:, 0:1], fcb_sb[:, 1:2])
            bd_ps = ps.tile([BPC, 1], f32, tag="bdps")
            nc.tensor.matmul(bd_ps[:, :], ones_sb[:, :], bd_sb[:, :])
            bd_bc = sb.tile([BPC, 1], f32, tag="bdbc")
            nc.vector.tensor_copy(bd_bc[:, :], bd_ps[:, :])

            # --- U0 - U1 = v @ W0 + (-v) @ W1, accumulated in PSUM ----------
            if mm_mode == "bf16":
                vt_p = sb.tile([128, NT * BPC], bf16, tag="vtb")
                nc.vector.tensor_copy(vt_p[:, :], vt_sb[:, :])
                vt_n = sb.tile([128, NT * BPC], bf16, tag="vtnb")
                nc.vector.tensor_copy(vt_n[:, :], vtn_sb[:, :])
                w_mm = []
                for k in range(2):
                    wb = sb.tile([128, NT * D], bf16, tag=f"wb{k}", name=f"wb{k}")
                    for c in range(WCHUNKS):
                        sl = slice(c * 2 * D, (c + 1) * 2 * D)
                        nc.vector.tensor_copy(wb[:, sl], w_sb[k][:, sl])
                    w_mm.append(wb)
                vt_k = [_APWrap(vt_p), _APWrap(vt_n)]
                w_mm = [_APWrap(w) for w in w_mm]
            else:
                mmdt = f32r if mm_mode == "f32r" else f32
                vt_k = [_APWrap(vt_sb.bitcast(mmdt)), _APWrap(vtn_sb.bitcast(mmdt))]
                w_mm = [_APWrap(w.bitcast(mmdt)) for w in w_sb]

            NW = D // NH  # 384
            u_ps = []
            for h in range(NH):
                u_ps.append(ps.tile([BPC, NW], f32, tag=f"u{h}", name=f"u{h}"))
            for k in range(2):
                for t in range(NT):
                    for h in range(NH):
                        nc.tensor.matmul(
                            u_ps[h][:, :],
                            vt_k[k][:, t * BPC : (t + 1) * BPC],
                            w_mm[k][:, t * D + h * NW : t * D + (h + 1) * NW],
                            start=(k == 0 and t == 0),
                            stop=(k == 1 and t == NT - 1),
                        )

            # --- delta = rowsum((U0-U1) * s) -------------------------------
            # (tensor_tensor_reduce crashes TRN2 HW via this stack; use
            # tensor_mul + reduce_sum instead)
            scr_sb = sb.tile([BPC, D], f32, tag="scr")
            delta_sb = sb.tile([BPC, 1], f32, tag="delta")
            for h in range(NH):
                nc.vector.tensor_mul(
                    scr_sb[:, h * NW : (h + 1) * NW],
                    u_ps[h][:, :],
                    s_sb[:, h * NW : (h + 1) * NW],
                )
            nc.vector.reduce_sum(delta_sb[:, :], scr_sb[:, :], mybir.AxisListType.X)

            # --- a = sigmoid(delta + (b0-b1)) ------------------------------
            a_sb = sb.tile([BPC, 1], f32, tag="a")
            nc.scalar.activation(
                a_sb[:, :], delta_sb[:, :], Act.Sigmoid, bias=bd_bc[:, :], scale=1.0
            )

            # --- out = s + a*(v-s) -----------------------------------------
            vms_sb = sb.tile([BPC, D], f32, tag="vms")
            nc.vector.tensor_sub(vms_sb[:, :], v_sb[:, :], s_sb[:, :])
            o_sb = sb.tile([BPC, D], f32, tag="o")
            nc.vector.scalar_tensor_tensor(
                o_sb[:, :],
                vms_sb[:, :],
                a_sb[:, :],
                s_sb[:, :],
                AluOp.mult,
                AluOp.add,
            )
            nc.sync.dma_start(out=out_ext[:, :], in_=o_sb[:, :])

    nc.compile()
    return nc


class _APWrap:
    """Slice helper so bitcast whole-tensor APs can be sliced like tiles."""

    def __init__(self, ap):
        self._ap = ap

    def __getitem__(self, idx):
        return self._ap[idx]


def kernel(v_x, s_x, fc_w, fc_b):
    from concourse.bass_utils import run_bass_kernel_spmd

    key = "nc"
    if key not in _CACHE:
        _CACHE[key] = _build()
    nc = _CACHE[key]

    v_x = np.ascontiguousarray(v_x, dtype=np.float32)
    s_x = np.ascontiguousarray(s_x, dtype=np.float32)
    fc_w = np.ascontiguousarray(fc_w, dtype=np.float32)
    fc_b = np.ascontiguousarray(fc_b, dtype=np.float32)
    ident = np.eye(BPC, dtype=np.float32)

    in_maps = []
    for m in range(NCORES):
        rows = slice(m * BPC, (m + 1) * BPC)
        in_maps.append(
            {
                "v_x": v_x[rows],
                "s_x": s_x[rows],
                "fc_w": fc_w,
                "fc_b": fc_b,
                "ident": ident,
            }
        )

    res = run_bass_kernel_spmd(nc, in_maps, core_ids=list(range(NCORES)))
    out = np.concatenate([res.results[m]["out"] for m in range(NCORES)], axis=0)
    return out.astype(np.float32)


if __name__ == "__main__":
    rng = np.random.default_rng(0)
    v = rng.standard_normal((B, D), dtype=np.float32)
    s = rng.standard_normal((B, D), dtype=np.float32)
    w = (rng.standard_normal((2, D * D), dtype=np.float32) * 0.01).astype(np.float32)
    b = np.zeros((2,), dtype=np.float32)
    o = kernel(v_x=v, s_x=s, fc_w=w, fc_b=b)
    print(o.shape, o.dtype)


# revision 11
# speedup vs baseline: 1.1494x; 1.1494x over previous
"""AdaptiveFeatureFusion Trainium2 kernel (8 NeuronCores, data-parallel).

Math rewrite: softmax over 2 logits -> sigmoid of the logit difference.
  delta[b] = sum_ij v[b,i] * (W0 - W1)[i,j] * s[b,j] + (b0 - b1)
           = rowsum((v @ W0 - v @ W1) * s) + (b0 - b1)
  a[b]     = sigmoid(delta[b])
  out[b,:] = a[b] * v[b,:] + (1 - a[b]) * s[b,:] = s + a*(v - s)

Sharding: batch dim (512) split across 8 cores (64 rows each); the
(2, 768*768) fc weight is replicated and streamed through each core.
Per-core compute: vT via PE transposes, U_k = v_rows @ W_k on TensorE
(float32r, full rate at N>=384), the dot/sigmoid/fusion epilogue on
Vector/Scalar engines.
"""

import os
import sys

for _p in ("/opt/trn_rl_repo", "/opt/pypackages"):
    if os.path.isdir(_p) and _p not in sys.path:
        sys.path.append(_p)

import numpy as np

B = 512
D = 768
NCORES = 8
BPC = B // NCORES  # 64 rows per core
NT = D // 128  # 6 i-tiles
NH = 2  # N halves of 384
WCHUNKS = 3  # DMA chunks per W row (2 i-tiles each)

_CACHE = {}


def _build(mm_mode="f32"):
    from concourse import bacc, mybir
    from concourse import tile

    f32 = mybir.dt.float32
    f32r = mybir.dt.float32r
    bf16 = mybir.dt.bfloat16
    AluOp = mybir.AluOpType
    Act = mybir.ActivationFunctionType

    nc = bacc.Bacc(None, target_bir_lowering=False)

    v_ext = nc.declare_dram_parameter("v_x", [BPC, D], f32, isOutput=False)
    s_ext = nc.declare_dram_parameter("s_x", [BPC, D], f32, isOutput=False)
    w_ext = nc.declare_dram_parameter("fc_w", [2, D * D], f32, isOutput=False)
    b_ext = nc.declare_dram_parameter("fc_b", [2], f32, isOutput=False)
    id_ext = nc.declare_dram_parameter("ident", [BPC, BPC], f32, isOutput=False)
    out_ext = nc.declare_dram_parameter("out", [BPC, D], f32, isOutput=True)

    with tile.TileContext(nc) as tc:
        with (
            tc.tile_pool(name="sb", bufs=1) as sb,
            tc.tile_pool(name="ps", bufs=1, space="PSUM") as ps,
            tc.tile_pool(name="tps", bufs=2, space="PSUM") as tps,
        ):
            # --- small control DMAs first (so they don't queue behind W) ---
            id_sb = sb.tile([BPC, BPC], f32, tag="id")
            nc.sync.dma_start(out=id_sb[:, :], in_=id_ext[:, :])
            fcb_sb = sb.tile([1, 2], f32, tag="fcb")
            nc.sync.dma_start(out=fcb_sb[:, :], in_=b_ext.ap().unsqueeze(0))
            v_sb = sb.tile([BPC, D], f32, tag="v")
            nc.sync.dma_start(out=v_sb[:, :], in_=v_ext[:, :])
            s_sb = sb.tile([BPC, D], f32, tag="s")
            nc.sync.dma_start(out=s_sb[:, :], in_=s_ext[:, :])

            # --- vT via PE transposes, cast to bf16 in the PSUM->SBUF copy;
            # negated copy feeds the k=1 matmuls so U0 - U1 accumulates
            # directly in PSUM. ---------------------------------------------
            vt_p = sb.tile([128, NT * BPC], bf16, tag="vtp")
            vt_n = sb.tile([128, NT * BPC], bf16, tag="vtn")
            for t in range(NT):
                tp = tps.tile([128, BPC], f32, tag="tp")
                nc.tensor.transpose(
                    tp[:, :], v_sb[:, t * 128 : (t + 1) * 128], id_sb[:, :]
                )
                nc.vector.tensor_copy(vt_p[:, t * BPC : (t + 1) * BPC], tp[:, :])
                nc.vector.tensor_scalar_mul(
                    vt_n[:, t * BPC : (t + 1) * BPC], tp[:, :], -1.0
                )
            vt_k = [vt_p, vt_n]

            # --- bias difference broadcast to all partitions ---------------
            ones_sb = sb.tile([1, BPC], f32, tag="ones")
            nc.gpsimd.memset(ones_sb[:, :], 1.0)
            bd_sb = sb.tile([1, 1], f32, tag="bd")
            nc.vector.tensor_sub(bd_sb[:, :], fcb_sb[:, 0:1], fcb_sb[:, 1:2])
            bd_ps = ps.tile([BPC, 1], f32, tag="bdps")
            nc.tensor.matmul(bd_ps[:, :], ones_sb[:, :], bd_sb[:, :])
            bd_bc = sb.tile([BPC, 1], f32, tag="bdbc")
            nc.vector.tensor_copy(bd_bc[:, :], bd_ps[:, :])

            # --- W stream: per k one f32 landing tile (128, NT*768), free
            # dim is (i_tile, j) i_tile-major; DMA'd in WCHUNKS chunks, each
            # converted to bf16 on DVE as it lands. -------------------------
            w_sb = []
            wb_sb = []
            for k in range(2):
                w_sb.append(sb.tile([128, NT * D], f32, tag=f"w{k}", name=f"w{k}"))
                wb_sb.append(
                    sb.tile([128, NT * D], bf16, tag=f"wb{k}", name=f"wb{k}")
                )
            tpc = NT // WCHUNKS  # i-tiles per chunk
            rows_per_chunk = tpc * 128 * D
            for k in range(2):
                for c in range(WCHUNKS):
                    src = w_ext[k, c * rows_per_chunk : (c + 1) * rows_per_chunk]
                    src = src.rearrange("(t p j) -> p t j", t=tpc, p=128, j=D)
                    sl = slice(c * tpc * D, (c + 1) * tpc * D)
                    dst = w_sb[k][:, sl].rearrange("p (t j) -> p t j", t=tpc, j=D)
                    nc.sync.dma_start(out=dst, in_=src)
                    nc.vector.tensor_copy(wb_sb[k][:, sl], w_sb[k][:, sl])

            # --- U0 - U1 = v @ W0 + (-v) @ W1, accumulated in PSUM ----------
            NW = D // NH  # 384
            u_ps = []
            for h in range(NH):
                u_ps.append(ps.tile([BPC, NW], f32, tag=f"u{h}", name=f"u{h}"))
            for k in range(2):
                for t in range(NT):
                    for h in range(NH):
                        nc.tensor.matmul(
                            u_ps[h][:, :],
                            vt_k[k][:, t * BPC : (t + 1) * BPC],
                            wb_sb[k][:, t * D + h * NW : t * D + (h + 1) * NW],
                            start=(k == 0 and t == 0),
                            stop=(k == 1 and t == NT - 1),
                        )

            # --- delta = rowsum((U0-U1) * s) -------------------------------
            # (tensor_tensor_reduce crashes TRN2 HW via this stack; use
            # tensor_mul + reduce_sum instead)
            scr_sb = sb.tile([BPC, D], f32, tag="scr")
            delta_sb = sb.tile([BPC, 1], f32, tag="delta")
            for h in range(NH):
                nc.vector.tensor_mul(
                    scr_sb[:, h * NW : (h + 1) * NW],
                    u_ps[h][:, :],
                    s_sb[:, h * NW : (h + 1) * NW],
                )
            nc.vector.reduce_sum(delta_sb[:, :], scr_sb[:, :], mybir.AxisListType.X)

            # --- a = sigmoid(delta + (b0-b1)) ------------------------------
            a_sb = sb.tile([BPC, 1], f32, tag="a")
            nc.scalar.activation(
                a_sb[:, :], delta_sb[:, :], Act.Sigmoid, bias=bd_bc[:, :], scale=1.0
            )

            # --- out = s + a*(v-s) -----------------------------------------
            vms_sb = sb.tile([BPC, D], f32, tag="vms")
            nc.vector.tensor_sub(vms_sb[:, :], v_sb[:, :], s_sb[:, :])
            o_sb = sb.tile([BPC, D], f32, tag="o")
            nc.vector.scalar_tensor_tensor(
                o_sb[:, :],
                vms_sb[:, :],
                a_sb[:, :],
                s_sb[:, :],
                AluOp.mult,
                AluOp.add,
            )
            nc.sync.dma_start(out=out_ext[:, :], in_=o_sb[:, :])

    nc.compile()
    return nc


class _APWrap:
    """Slice helper so bitcast whole-tensor APs can be sliced like tiles."""

    def __init__(self, ap):
        self._ap = ap

    def __getitem__(self, idx):
        return self._ap[idx]


def kernel(v_x, s_x, fc_w, fc_b):
    from concourse.bass_utils import run_bass_kernel_spmd

    key = "nc"
    if key not in _CACHE:
        _CACHE[key] = _build()
    nc = _CACHE[key]

    v_x = np.ascontiguousarray(v_x, dtype=np.float32)
    s_x = np.ascontiguousarray(s_x, dtype=np.float32)
    fc_w = np.ascontiguousarray(fc_w, dtype=np.float32)
    fc_b = np.ascontiguousarray(fc_b, dtype=np.float32)
    ident = np.eye(BPC, dtype=np.float32)

    in_maps = []
    for m in range(NCORES):
        rows = slice(m * BPC, (m + 1) * BPC)
        in_maps.append(
            {
                "v_x": v_x[rows],
                "s_x": s_x[rows],
                "fc_w": fc_w,
                "fc_b": fc_b,
                "ident": ident,
            }
        )

    res = run_bass_kernel_spmd(nc, in_maps, core_ids=list(range(NCORES)))
    out = np.concatenate([res.results[m]["out"] for m in range(NCORES)], axis=0)
    return out.astype(np.float32)


if __name__ == "__main__":
    rng = np.random.default_rng(0)
    v = rng.standard_normal((B, D), dtype=np.float32)
    s = rng.standard_normal((B, D), dtype=np.float32)
    w = (rng.standard_normal((2, D * D), dtype=np.float32) * 0.01).astype(np.float32)
    b = np.zeros((2,), dtype=np.float32)
    o = kernel(v_x=v, s_x=s, fc_w=w, fc_b=b)
    print(o.shape, o.dtype)
